# revision 35
# baseline (speedup 1.0000x reference)
"""DeepSeek-MoE Trainium2 kernel (8 NeuronCores, expert-parallel).

Strategy
--------
* Routing (sigmoid + grouped top-k, DeepSeek noaux_tc) is computed on the
  HOST in fp32 (exact mirror of the reference ops). The device consumes the
  routing results as dense inputs: a one-hot dispatch matrix D[t, c] and the
  transposed combine-weight matrix WcT[c, t] (weights * RSF, renormalized).
  This removes the fp32 x load, the logits GEMM and the whole on-device
  routing chain from the kernel.
* Expert parallelism: 4 experts per core, assigned rank-strided by load so
  the per-slot max capacity across cores (the SPMD program is shared) stays
  tight. Capacities are exact loads rounded to 8 (no 128 padding).
* Expert GEMMs are WEIGHT-STATIONARY: stationary = weight tile [k, 128],
  moving = activations [k, cap_e]. PE matmul cost is out_cols x k_tiles
  regardless of partition fill, so exact (non-128-padded) capacities cut
  ~33% of the expert-GEMM PE time vs. token-stationary tiles.
* Shared experts are sharded over the intermediate dim (352 ch/core) and run
  f-major (stationary = wgu tile) so wgu streams exactly once and the
  activations land directly in [i_s, t] layout for the combine.
* Everything on device is bf16 except PSUM accumulation; output partials are
  bf16 (host upcasts to fp32 after the ReduceScatter).
* Schedule: one SP-queue DMA stream (x, D, WcT, wgu, wdn, then w13/w2 chunks
  in consumption order with lookahead); PE head = x-transposes + dispatch;
  shared-expert chains, combine passes and ye-transposes are WOVEN between
  expert weight-chunk consumption to keep PE continuously busy (p-state) while
  DMA (the roofline, ~70MB of expert weights) never stalls.
"""

import numpy as np
import ml_dtypes

T, H, E, K, I = 512, 2048, 32, 8, 1408
NG, TKG = 8, 4
RSF = 2.5
C = 2 * T * K // E          # 256 per-expert capacity
NCORES = 8
P = 128
ISH = 2 * I // NCORES       # 352 shared-intermediate slice per core
HT = H // P                 # 16
TT = T // P                 # 4
ITL = I // P                # 11
GS = E // NG                # 4

bf16 = ml_dtypes.bfloat16


# ---------------------------------------------------------------------------
# Host routing (exact fp32 mirror of reference._route)
# ---------------------------------------------------------------------------
def _host_route(x, gate_w, bias):
    logits = x.astype(np.float32) @ gate_w.astype(np.float32)
    scores = (1.0 / (1.0 + np.exp(-logits.astype(np.float32)))).astype(np.float32)
    sb = scores + bias[None, :].astype(np.float32)
    g = sb.reshape(T, NG, GS)
    gs = np.sort(g, axis=-1)
    grp = gs[..., -1] + gs[..., -2]
    gidx = np.argsort(-grp, axis=-1, kind="stable")[:, :TKG]
    gmask = np.zeros((T, NG), bool)
    gmask[np.arange(T)[:, None], gidx] = True
    masked = np.where(gmask[:, :, None], g, -np.inf).reshape(T, E)
    topk = np.argsort(-masked, axis=-1, kind="stable")[:, :K]
    w = np.take_along_axis(scores, topk, axis=1)
    w = w / w.sum(-1, keepdims=True)
    return (w * RSF).astype(np.float32), topk.astype(np.int64)


def _plan(topk):
    loads = np.bincount(topk.reshape(-1), minlength=E)
    order = np.argsort(-loads, kind="stable")
    groups = [[int(order[j * NCORES + c]) for j in range(4)]
              for c in range(NCORES)]
    slot_caps = []
    for j in range(4):
        mx = max(min(int(loads[order[j * NCORES + c]]), C)
                 for c in range(NCORES))
        slot_caps.append(max(8, int(np.ceil(mx / 8) * 8)))
    return groups, slot_caps


def _build_dispatch(x, weights, topk, groups, slot_caps):
    """Per-core dispatched activations xeT [P, HT, DCOLS] (bf16 of x rows,
    the D^T @ x gather done host-side) and WcT [P, NCB, T] fp32."""
    offs = np.cumsum([0] + slot_caps)
    DCOLS = int(offs[-1])
    NCB = (DCOLS + P - 1) // P
    flat_e = topk.reshape(-1)
    tok = np.repeat(np.arange(T), K)
    wf = weights.reshape(-1)
    xb = x.astype(bf16)
    xeTs, WcTs = [], []
    for core in range(NCORES):
        xe = np.zeros((DCOLS, H), bf16)
        Wc = np.zeros((T, NCB * P), np.float32)
        for j, e in enumerate(groups[core]):
            pos = np.flatnonzero(flat_e == e)[:C]
            r = np.arange(len(pos))
            xe[offs[j] + r] = xb[tok[pos]]
            Wc[tok[pos], offs[j] + r] = wf[pos]
        xeT = np.ascontiguousarray(
            xe.T.reshape(HT, P, DCOLS).transpose(1, 0, 2))  # [P, HT, DCOLS]
        WcT = np.ascontiguousarray(
            Wc.reshape(T, NCB, P).transpose(2, 1, 0))       # [P, NCB, T]
        xeTs.append(xeT)
        WcTs.append(WcT)
    return xeTs, WcTs, offs, DCOLS, NCB


# ---------------------------------------------------------------------------
# Host weight packing
# ---------------------------------------------------------------------------
def _pack_w13(w):
    """w [H, 2I] -> [P, ITL, HT*2P]: chunk i holds (g_i | u_i) per h-tile."""
    out = np.empty((P, ITL, HT * 2 * P), bf16)
    for i in range(ITL):
        for k in range(HT):
            blk = np.empty((P, 2 * P), np.float32)
            blk[:, :P] = w[k * P:(k + 1) * P, i * P:(i + 1) * P]
            blk[:, P:] = w[k * P:(k + 1) * P, I + i * P:I + (i + 1) * P]
            out[:, i, k * 2 * P:(k + 1) * 2 * P] = blk.astype(bf16)
    return out


def _pack_w2(w):
    """w [I, H] -> [P, 8, ITL*2P]: chunk hc holds h-tiles (2hc, 2hc+1)."""
    out = np.empty((P, 8, ITL * 2 * P), bf16)
    for hc in range(8):
        for ki in range(ITL):
            blk = np.empty((P, 2 * P), np.float32)
            blk[:, :P] = w[ki * P:(ki + 1) * P, (2 * hc) * P:(2 * hc + 1) * P]
            blk[:, P:] = w[ki * P:(ki + 1) * P, (2 * hc + 1) * P:(2 * hc + 2) * P]
            out[:, hc, ki * 2 * P:(ki + 1) * 2 * P] = blk.astype(bf16)
    return out


def _pack_wgu(sgu, core):
    """[H, 2*2816] -> [P, HT, 704] cols [g0|u0|g1|u1|g2(96)|u2(96)]."""
    gsl = sgu[:, core * ISH:(core + 1) * ISH]
    usl = sgu[:, 2 * I + core * ISH:2 * I + (core + 1) * ISH]
    segs = [(0, 128), (128, 256), (256, 352)]
    out = np.zeros((P, HT, 704), np.float32)
    for k in range(HT):
        col = 0
        for (a, b) in segs:
            w = b - a
            out[:, k, col:col + w] = gsl[k * P:(k + 1) * P, a:b]
            col += w
            out[:, k, col:col + w] = usl[k * P:(k + 1) * P, a:b]
            col += w
    return out.astype(bf16)


def _pack_wdn(sdn, core):
    out = np.zeros((P, 3, H), np.float32)
    sl = sdn[core * ISH:(core + 1) * ISH, :]
    for s in range(3):
        r = min(P, ISH - s * P)
        out[:r, s, :] = sl[s * P:s * P + r, :]
    return out.astype(bf16)


# ---------------------------------------------------------------------------
# Device program
# ---------------------------------------------------------------------------
def _build_nc(slot_caps, single_core=False):
    import concourse.mybir as mybir
    import concourse.tile as tile
    from concourse import bacc
    from contextlib import ExitStack
    from collections import deque

    f32 = mybir.dt.float32
    b16 = mybir.dt.bfloat16
    Alu = mybir.AluOpType
    Act = mybir.ActivationFunctionType

    offs = np.cumsum([0] + list(slot_caps)).tolist()
    DCOLS = offs[-1]
    NCB = (DCOLS + P - 1) // P
    CAPMAX = max(slot_caps)
    YRING = 3

    nc = bacc.Bacc("TRN2", target_bir_lowering=False, debug=False,
                   num_devices=1 if single_core else NCORES)

    xet_d = nc.dram_tensor("xet", [P, HT, DCOLS], b16, kind="ExternalInput")
    xt_d = nc.dram_tensor("xt", [P, HT, T], b16, kind="ExternalInput")
    wct_d = nc.dram_tensor("wct", [P, NCB, T], b16, kind="ExternalInput")
    w13_d = nc.dram_tensor("w13p", [4, P, ITL * HT * 2 * P], b16,
                           kind="ExternalInput")
    w2_d = nc.dram_tensor("w2p", [4, P, 8 * ITL * 2 * P], b16,
                          kind="ExternalInput")
    wgu_d = nc.dram_tensor("wgup", [P, HT, 704], b16, kind="ExternalInput")
    wdn_d = nc.dram_tensor("wdnp", [P, 3, H], b16, kind="ExternalInput")
    id16_d = nc.dram_tensor("id_b16", [P, P], b16, kind="ExternalInput")
    out_d = nc.dram_tensor("out_slice",
                           [T, H] if single_core else [T // NCORES, H], b16,
                           kind="ExternalOutput")
    if not single_core:
        partial_d = nc.dram_tensor("partial", [T, H], b16, kind="Internal")
        rs_d = nc.dram_tensor("rs_out", [T // NCORES, H], b16, kind="Internal")

    # shared-gu packed col segments: (colofs, width) pairs g/u interleaved
    SSEG = [(0, 128), (256, 128), (512, 96)]       # gate col starts in wgu_p

    with tile.TileContext(nc) as tc, ExitStack() as ctx:
        pc = ctx.enter_context(tc.tile_pool(name="persist", bufs=1))
        w13sp = ctx.enter_context(tc.tile_pool(name="w13s", bufs=4))
        w2sp = ctx.enter_context(tc.tile_pool(name="w2s", bufs=3))
        ap_ = ctx.enter_context(tc.tile_pool(name="acts", bufs=2))
        tp_ = ctx.enter_context(tc.tile_pool(name="tmps", bufs=2))
        psS = ctx.enter_context(tc.tile_pool(name="psumS", bufs=2, space="PSUM"))
        psG = ctx.enter_context(tc.tile_pool(name="psumG", bufs=2, space="PSUM"))
        psU = ctx.enter_context(tc.tile_pool(name="psumU", bufs=2, space="PSUM"))
        psSG = ctx.enter_context(tc.tile_pool(name="psumSG", bufs=1, space="PSUM"))

        # ---- input DMAs in arrival order (SP queue) ----
        id16_sb = pc.tile([P, P], b16, tag="id16")
        nc.sync.dma_start(id16_sb[:], id16_d.ap())
        xeT = pc.tile([P, HT, DCOLS], b16, tag="xeT")
        nc.sync.dma_start(xeT[:], xet_d.ap())
        xT = pc.tile([P, HT, T], b16, tag="xT")
        nc.sync.dma_start(xT[:], xt_d.ap())
        wgu_sb = pc.tile([P, HT, 704], b16, tag="wgu")
        nc.sync.dma_start(wgu_sb[:], wgu_d.ap())
        wdn_sb = pc.tile([P, 3, H], b16, tag="wdn")
        nc.sync.dma_start(wdn_sb[:], wdn_d.ap())
        wct_sb = pc.tile([P, NCB, T], b16, tag="wct")
        nc.sync.dma_start(wct_sb[:], wct_d.ap())

        # ---- weight stream: consumption-order chunk list with lookahead ----
        stream = []
        for j in range(4):
            stream += [("w13", j, i) for i in range(ITL)]
            stream += [("w2", j, hc) for hc in range(8)]
        BUFS = {"w13": 4, "w2": 3}
        tiles = {}
        outst = {"w13": 0, "w2": 0}
        nxt = [0]

        def issue_avail():
            # issue in stream order while pool capacity is free, so a full
            # pool never parks a blocking DMA at the queue head
            while nxt[0] < len(stream):
                kind, j, i = stream[nxt[0]]
                if outst[kind] >= BUFS[kind]:
                    return
                if kind == "w13":
                    tl = w13sp.tile([P, HT * 2 * P], b16, tag="w13c",
                                    name="w13c")
                    nc.sync.dma_start(
                        tl[:],
                        w13_d.ap()[j][:, i * HT * 2 * P:(i + 1) * HT * 2 * P])
                else:
                    tl = w2sp.tile([P, ITL * 2 * P], b16, tag="w2c",
                                   name="w2c")
                    nc.sync.dma_start(
                        tl[:],
                        w2_d.ap()[j][:, i * ITL * 2 * P:(i + 1) * ITL * 2 * P])
                tiles[nxt[0]] = tl
                outst[kind] += 1
                nxt[0] += 1

        issue_avail()
        consumed = [0]

        def take():
            n = consumed[0]
            tl = tiles.pop(n)
            consumed[0] = n + 1
            outst[stream[n][0]] -= 1
            issue_avail()
            return tl

        # ---- persistent activations ----
        actShT = pc.tile([P, 3, T], b16, tag="actShT")
        peS = pc.tile([P, TT, H], b16, tag="peS")
        ye = pc.tile([P, YRING, H], b16, tag="ye")
        yeT = pc.tile([P, HT, NCB * P], b16, tag="yeT")
        if NCB * P > DCOLS:
            nc.gpsimd.memset(yeT[:, :, DCOLS:], 0.0)

        cpi = [0]

        def cp(out, in_):
            if cpi[0] % 2 == 0:
                nc.vector.tensor_copy(out=out, in_=in_)
            else:
                nc.scalar.copy(out, in_)
            cpi[0] += 1

        # ---- woven PE work: shared-expert chains + combine passes ----
        pending = deque()

        def weave(k=1):
            for _ in range(k):
                if pending:
                    pending.popleft()()

        def shared_pair(mb):
            # 4 pieces of ~1.7us: gate/up chains each split into column halves
            gcol, w = SSEG[mb]
            hold = {}

            def chain(ps, c0, c1, wofs):
                for k in range(HT):
                    nc.tensor.matmul(ps[:w, c0:c1],
                                     wgu_sb[:, k, gcol + wofs:gcol + wofs + w],
                                     xT[:, k, c0:c1],
                                     start=(k == 0), stop=(k == HT - 1))

            def g_half(half):
                if half == 0:
                    hold["sg"] = psSG.tile([P, 512], f32, tag="sg", name="ps_g")
                chain(hold["sg"], half * 256, half * 256 + 256, 0)
                if half == 1:
                    tmp = tp_.tile([P, 512], b16, tag="sgtmp", name="sgtmp",
                                   bufs=1)
                    nc.scalar.activation(tmp[:w, :], hold["sg"][:w, :], Act.Silu)
                    hold["tmp"] = tmp

            def u_half(half):
                if half == 0:
                    hold["su"] = psSG.tile([P, 512], f32, tag="su", name="ps_u")
                chain(hold["su"], half * 256, half * 256 + 256, w)
                if half == 1:
                    nc.vector.tensor_tensor(actShT[:w, mb, :],
                                            hold["tmp"][:w, :],
                                            hold["su"][:w, :], Alu.mult)

            pending.append(lambda: g_half(0))
            pending.append(lambda: g_half(1))
            pending.append(lambda: u_half(0))
            pending.append(lambda: u_half(1))

        pass_first = [True]
        cpool = [0]

        def cpiece(tt, hq, cbs, with_shared, first, last):
            pool, tag = (psG, "g") if cpool[0] % 2 == 0 else (psU, "u")
            cpool[0] += 1
            po = pool.tile([P, 512], f32, tag=tag, name="po")
            nmm = (3 if with_shared else 0) + len(cbs)
            q = 0
            if with_shared:
                for s in range(3):
                    r = min(P, ISH - s * P)
                    nc.tensor.matmul(
                        po[:], actShT[:r, s, tt * P:(tt + 1) * P],
                        wdn_sb[:r, s, hq * 512:(hq + 1) * 512],
                        start=(q == 0), stop=(q == nmm - 1))
                    q += 1
            for cb in cbs:
                nc.tensor.matmul(
                    po[:], wct_sb[:, cb, tt * P:(tt + 1) * P],
                    ye[:, cb % YRING, hq * 512:(hq + 1) * 512],
                    start=(q == 0), stop=(q == nmm - 1))
                q += 1
            dst = peS[:, tt, hq * 512:(hq + 1) * 512]
            if first:
                nc.vector.tensor_copy(out=dst, in_=po[:])
            elif not last:
                nc.vector.tensor_tensor(dst, dst, po[:], Alu.add)
            else:
                stg = tp_.tile([P, 512], b16, tag="stg", name="stg", bufs=4)
                nc.vector.tensor_tensor(stg[:], dst, po[:], Alu.add)
                if single_core:
                    dd = out_d.ap()[tt * P:(tt + 1) * P,
                                    hq * 512:(hq + 1) * 512]
                else:
                    dd = partial_d.ap()[tt * P:(tt + 1) * P,
                                        hq * 512:(hq + 1) * 512]
                nc.gpsimd.dma_start(dd, stg[:])

        def combine_pass(cbs, with_shared):
            if not cbs and not with_shared:
                return
            first = pass_first[0]
            for tt in range(TT):
                for hq in range(4):
                    pending.append(
                        lambda tt=tt, hq=hq, cbs=tuple(cbs), ws=with_shared,
                        first=first: cpiece(tt, hq, cbs, ws, first, False))
            pass_first[0] = False

        # ---- expert pipeline ----
        cb_done = [0]

        def ye_transpose(cb, s):
            pt = psS.tile([P, 2 * P], b16, tag="sm", name="pt_ye")
            for hl in range(2):
                nc.tensor.transpose(
                    pt[:, hl * P:(hl + 1) * P],
                    yeT[:, 2 * s + hl, cb * P:(cb + 1) * P], id16_sb[:])
            cp(ye[:, cb % YRING, 2 * s * P:(2 * s + 2) * P],
               pt.rearrange("p (k c) -> p k c", k=2))

        def expert(j):
            cap = slot_caps[j]
            off = offs[j]
            last = (j == 3)
            if j < 3:
                shared_pair(j)
            act = ap_.tile([P, ITL, CAPMAX], b16, tag="act", name="act")
            for i in range(ITL):
                wg = take().rearrange("p (k f) -> p k f", f=2 * P)
                pg = psG.tile([P, 512], f32, tag="g", name="pg_w13")
                pu = psU.tile([P, 512], f32, tag="u", name="pu_w13")
                for k in range(HT):
                    nc.tensor.matmul(pg[:, :cap], wg[:, k, :P],
                                     xeT[:, k, off:off + cap],
                                     start=(k == 0), stop=(k == HT - 1))
                    nc.tensor.matmul(pu[:, :cap], wg[:, k, P:2 * P],
                                     xeT[:, k, off:off + cap],
                                     start=(k == 0), stop=(k == HT - 1))
                tmp = tp_.tile([P, 512], b16, tag="silu", name="silu")
                nc.scalar.activation(tmp[:, :cap], pg[:, :cap], Act.Silu)
                nc.vector.tensor_tensor(act[:, i, :cap], tmp[:, :cap],
                                        pu[:, :cap], Alu.mult)
                weave(2 if last else 1)
            hi = (offs[j + 1] // P) if j < 3 else NCB
            cbs = list(range(cb_done[0], hi))
            cb_done[0] = hi
            if last:
                while pending:       # all earlier peS adds must precede stg
                    pending.popleft()()
            for hc in range(8):
                w2c = take().rearrange("p (k f) -> p k f", f=2 * P)
                for m in range(2):
                    pool = psG if m == 0 else psU
                    py = pool.tile([P, 512], f32, tag="g" if m == 0 else "u",
                                   name="py")
                    for ki in range(ITL):
                        nc.tensor.matmul(py[:, :cap], w2c[:, ki, m * P:(m + 1) * P],
                                         act[:, ki, :cap],
                                         start=(ki == 0), stop=(ki == ITL - 1))
                    cp(yeT[:, 2 * hc + m, off:off + cap], py[:, :cap])
                if not last:
                    weave(1)
                else:
                    # h-interleaved finish: transpose this h-pair for the
                    # final c-blocks, then emit the output pieces for the
                    # completed h-quarter (overlaps the DMA stream tail)
                    for cb in cbs:
                        ye_transpose(cb, hc)
                    if hc % 2 == 1:
                        hq = hc // 2
                        for tt in range(TT):
                            cpiece(tt, hq, cbs, False, False, True)
            if not last:
                for cb in cbs:
                    for s in range(HT // 2):
                        ye_transpose(cb, s)
                        if s % 4 == 3:
                            weave(1)
                combine_pass(cbs, with_shared=(j == 2))

        for j in range(4):
            expert(j)
        while pending:
            pending.popleft()()

        if not single_core:
            nc.gpsimd.collective_compute(
                "ReduceScatter", Alu.add,
                replica_groups=[list(range(NCORES))],
                ins=[partial_d.ap().opt()],
                outs=[rs_d.ap().opt()],
            )
            nc.sync.dma_start(out_d.ap(), rs_d.ap())

    nc.compile()
    return nc


_NC_CACHE = {}


def _prepare(hidden_states, gate_w, bias):
    x = np.ascontiguousarray(np.asarray(hidden_states, np.float32))
    weights, topk = _host_route(x, np.ascontiguousarray(np.asarray(gate_w, np.float32)),
                                np.asarray(bias, np.float32))
    groups, slot_caps = _plan(topk)
    return x, weights, topk, groups, slot_caps


def kernel(hidden_states, residual, gate_w, bias, w13, w2, shared_gate_up,
           shared_down):
    from concourse.bass_utils import run_bass_kernel_spmd

    x, weights, topk, groups, slot_caps = _prepare(hidden_states, gate_w, bias)
    w13 = np.asarray(w13, np.float32)
    w2 = np.asarray(w2, np.float32)
    sgu = np.asarray(shared_gate_up, np.float32)
    sdn = np.asarray(shared_down, np.float32)

    xeTs, WcTs, offs, DCOLS, NCB = _build_dispatch(x, weights, topk, groups,
                                                   slot_caps)

    key = tuple(slot_caps)
    if key not in _NC_CACHE:
        _NC_CACHE[key] = _build_nc(slot_caps)
    nc = _NC_CACHE[key]

    id16 = np.eye(P, dtype=np.float32).astype(bf16)
    xt = np.ascontiguousarray(
        x.astype(bf16).T.reshape(HT, P, T).transpose(1, 0, 2))
    in_maps = []
    for core in range(NCORES):
        in_maps.append({
            "xet": xeTs[core],
            "xt": xt,
            "wct": WcTs[core].astype(bf16),
            "w13p": np.stack([_pack_w13(w13[e]).reshape(P, -1)
                              for e in groups[core]]),
            "w2p": np.stack([_pack_w2(w2[e]).reshape(P, -1)
                             for e in groups[core]]),
            "wgup": _pack_wgu(sgu, core),
            "wdnp": _pack_wdn(sdn, core),
            "id_b16": id16,
        })
    res = run_bass_kernel_spmd(nc, in_maps, core_ids=list(range(NCORES)))
    out = np.concatenate([np.asarray(res.results[c]["out_slice"])
                          for c in range(NCORES)], axis=0)
    return out.astype(np.float32)


# revision 46
# speedup vs baseline: 1.0166x; 1.0166x over previous
"""DeepSeek-MoE Trainium2 kernel (8 NeuronCores, expert-parallel).

Strategy
--------
* Routing (sigmoid + grouped top-k, DeepSeek noaux_tc) is computed on the
  HOST in fp32 (exact mirror of the reference ops). The device consumes the
  routing results as dense inputs: a one-hot dispatch matrix D[t, c] and the
  transposed combine-weight matrix WcT[c, t] (weights * RSF, renormalized).
  This removes the fp32 x load, the logits GEMM and the whole on-device
  routing chain from the kernel.
* Expert parallelism: 4 experts per core, assigned rank-strided by load so
  the per-slot max capacity across cores (the SPMD program is shared) stays
  tight. Capacities are exact loads rounded to 8 (no 128 padding).
* Expert GEMMs are WEIGHT-STATIONARY: stationary = weight tile [k, 128],
  moving = activations [k, cap_e]. PE matmul cost is out_cols x k_tiles
  regardless of partition fill, so exact (non-128-padded) capacities cut
  ~33% of the expert-GEMM PE time vs. token-stationary tiles.
* Shared experts are sharded over the intermediate dim (352 ch/core) and run
  f-major (stationary = wgu tile) so wgu streams exactly once and the
  activations land directly in [i_s, t] layout for the combine.
* Everything on device is bf16 except PSUM accumulation; output partials are
  bf16 (host upcasts to fp32 after the ReduceScatter).
* Schedule: one SP-queue DMA stream (x, D, WcT, wgu, wdn, then w13/w2 chunks
  in consumption order with lookahead); PE head = x-transposes + dispatch;
  shared-expert chains, combine passes and ye-transposes are WOVEN between
  expert weight-chunk consumption to keep PE continuously busy (p-state) while
  DMA (the roofline, ~70MB of expert weights) never stalls.
"""

import numpy as np
import ml_dtypes

T, H, E, K, I = 512, 2048, 32, 8, 1408
NG, TKG = 8, 4
RSF = 2.5
C = 2 * T * K // E          # 256 per-expert capacity
NCORES = 8
P = 128
ISH = 2 * I // NCORES       # 352 shared-intermediate slice per core
HT = H // P                 # 16
TT = T // P                 # 4
ITL = I // P                # 11
GS = E // NG                # 4

bf16 = ml_dtypes.bfloat16


# ---------------------------------------------------------------------------
# Host routing (exact fp32 mirror of reference._route)
# ---------------------------------------------------------------------------
def _host_route(x, gate_w, bias):
    logits = x.astype(np.float32) @ gate_w.astype(np.float32)
    scores = (1.0 / (1.0 + np.exp(-logits.astype(np.float32)))).astype(np.float32)
    sb = scores + bias[None, :].astype(np.float32)
    g = sb.reshape(T, NG, GS)
    gs = np.sort(g, axis=-1)
    grp = gs[..., -1] + gs[..., -2]
    gidx = np.argsort(-grp, axis=-1, kind="stable")[:, :TKG]
    gmask = np.zeros((T, NG), bool)
    gmask[np.arange(T)[:, None], gidx] = True
    masked = np.where(gmask[:, :, None], g, -np.inf).reshape(T, E)
    topk = np.argsort(-masked, axis=-1, kind="stable")[:, :K]
    w = np.take_along_axis(scores, topk, axis=1)
    w = w / w.sum(-1, keepdims=True)
    return (w * RSF).astype(np.float32), topk.astype(np.int64)


def _plan(topk):
    loads = np.bincount(topk.reshape(-1), minlength=E)
    order = np.argsort(-loads, kind="stable")
    groups = [[int(order[j * NCORES + c]) for j in range(4)]
              for c in range(NCORES)]
    slot_caps = []
    for j in range(4):
        mx = max(min(int(loads[order[j * NCORES + c]]), C)
                 for c in range(NCORES))
        slot_caps.append(max(8, int(np.ceil(mx / 8) * 8)))
    return groups, slot_caps


def _build_dispatch(weights, topk, groups, slot_caps):
    """Per-core one-hot dispatch D [T, DCOLS] and WcT [P, NCB, T] fp32."""
    offs = np.cumsum([0] + slot_caps)
    DCOLS = int(offs[-1])
    NCB = (DCOLS + P - 1) // P
    flat_e = topk.reshape(-1)
    tok = np.repeat(np.arange(T), K)
    wf = weights.reshape(-1)
    Ds, WcTs = [], []
    for core in range(NCORES):
        D = np.zeros((T, DCOLS), np.float32)
        Wc = np.zeros((T, NCB * P), np.float32)
        for j, e in enumerate(groups[core]):
            pos = np.flatnonzero(flat_e == e)[:C]
            r = np.arange(len(pos))
            D[tok[pos], offs[j] + r] = 1.0
            Wc[tok[pos], offs[j] + r] = wf[pos]
        WcT = np.ascontiguousarray(
            Wc.reshape(T, NCB, P).transpose(2, 1, 0))       # [P, NCB, T]
        Ds.append(D)
        WcTs.append(WcT)
    return Ds, WcTs, offs, DCOLS, NCB


# ---------------------------------------------------------------------------
# Host weight packing
# ---------------------------------------------------------------------------
def _pack_w13(w):
    """w [H, 2I] -> [P, ITL, HT*2P]: chunk i holds (g_i | u_i) per h-tile."""
    out = np.empty((P, ITL, HT * 2 * P), bf16)
    for i in range(ITL):
        for k in range(HT):
            blk = np.empty((P, 2 * P), np.float32)
            blk[:, :P] = w[k * P:(k + 1) * P, i * P:(i + 1) * P]
            blk[:, P:] = w[k * P:(k + 1) * P, I + i * P:I + (i + 1) * P]
            out[:, i, k * 2 * P:(k + 1) * 2 * P] = blk.astype(bf16)
    return out


def _pack_w2(w):
    """w [I, H] -> [P, 8, ITL*2P]: chunk hc holds h-tiles (2hc, 2hc+1)."""
    out = np.empty((P, 8, ITL * 2 * P), bf16)
    for hc in range(8):
        for ki in range(ITL):
            blk = np.empty((P, 2 * P), np.float32)
            blk[:, :P] = w[ki * P:(ki + 1) * P, (2 * hc) * P:(2 * hc + 1) * P]
            blk[:, P:] = w[ki * P:(ki + 1) * P, (2 * hc + 1) * P:(2 * hc + 2) * P]
            out[:, hc, ki * 2 * P:(ki + 1) * 2 * P] = blk.astype(bf16)
    return out


def _pack_wgu(sgu, core):
    """[H, 2*2816] -> [P, HT, 704] cols [g0|u0|g1|u1|g2(96)|u2(96)]."""
    gsl = sgu[:, core * ISH:(core + 1) * ISH]
    usl = sgu[:, 2 * I + core * ISH:2 * I + (core + 1) * ISH]
    segs = [(0, 128), (128, 256), (256, 352)]
    out = np.zeros((P, HT, 704), np.float32)
    for k in range(HT):
        col = 0
        for (a, b) in segs:
            w = b - a
            out[:, k, col:col + w] = gsl[k * P:(k + 1) * P, a:b]
            col += w
            out[:, k, col:col + w] = usl[k * P:(k + 1) * P, a:b]
            col += w
    return out.astype(bf16)


def _pack_wdn(sdn, core):
    out = np.zeros((P, 3, H), np.float32)
    sl = sdn[core * ISH:(core + 1) * ISH, :]
    for s in range(3):
        r = min(P, ISH - s * P)
        out[:r, s, :] = sl[s * P:s * P + r, :]
    return out.astype(bf16)


# ---------------------------------------------------------------------------
# Device program
# ---------------------------------------------------------------------------
def _build_nc(slot_caps, single_core=False):
    import concourse.mybir as mybir
    import concourse.tile as tile
    from concourse import bacc
    from contextlib import ExitStack
    from collections import deque

    f32 = mybir.dt.float32
    b16 = mybir.dt.bfloat16
    Alu = mybir.AluOpType
    Act = mybir.ActivationFunctionType

    offs = np.cumsum([0] + list(slot_caps)).tolist()
    DCOLS = offs[-1]
    NCB = (DCOLS + P - 1) // P
    CAPMAX = max(slot_caps)
    cw0 = ((DCOLS // 2) + 7) // 8 * 8
    cw1 = DCOLS - cw0
    YRING = 3

    nc = bacc.Bacc("TRN2", target_bir_lowering=False, debug=False,
                   num_devices=1 if single_core else NCORES)

    x_d = nc.dram_tensor("x_bf", [T, H], b16, kind="ExternalInput")
    d_d = nc.dram_tensor("disp", [T, DCOLS], b16, kind="ExternalInput")
    wct_d = nc.dram_tensor("wct", [P, NCB, T], b16, kind="ExternalInput")
    w13_d = nc.dram_tensor("w13p", [4, P, ITL * HT * 2 * P], b16,
                           kind="ExternalInput")
    w2_d = nc.dram_tensor("w2p", [4, P, 8 * ITL * 2 * P], b16,
                          kind="ExternalInput")
    wgu_d = nc.dram_tensor("wgup", [P, HT, 704], b16, kind="ExternalInput")
    wdn_d = nc.dram_tensor("wdnp", [P, 3, H], b16, kind="ExternalInput")
    id16_d = nc.dram_tensor("id_b16", [P, P], b16, kind="ExternalInput")
    out_d = nc.dram_tensor("out_slice",
                           [T, H] if single_core else [T // NCORES, H], b16,
                           kind="ExternalOutput")
    if not single_core:
        partial_d = nc.dram_tensor("partial", [T, H], b16, kind="Internal")
        rs_d = nc.dram_tensor("rs_out", [T // NCORES, H], b16, kind="Internal")

    xr = x_d.ap().rearrange("(tt p) h -> p tt h", p=P)
    dr = d_d.ap().rearrange("(tt p) c -> p tt c", p=P)

    # shared-gu packed col segments: (colofs, width) pairs g/u interleaved
    SSEG = [(0, 128), (256, 128), (512, 96)]       # gate col starts in wgu_p

    with tile.TileContext(nc) as tc, ExitStack() as ctx:
        pc = ctx.enter_context(tc.tile_pool(name="persist", bufs=1))
        w13sp = ctx.enter_context(tc.tile_pool(name="w13s", bufs=4))
        w2sp = ctx.enter_context(tc.tile_pool(name="w2s", bufs=3))
        ap_ = ctx.enter_context(tc.tile_pool(name="acts", bufs=2))
        tp_ = ctx.enter_context(tc.tile_pool(name="tmps", bufs=2))
        psS = ctx.enter_context(tc.tile_pool(name="psumS", bufs=2, space="PSUM"))
        psG = ctx.enter_context(tc.tile_pool(name="psumG", bufs=2, space="PSUM"))
        psU = ctx.enter_context(tc.tile_pool(name="psumU", bufs=2, space="PSUM"))
        psSG = ctx.enter_context(tc.tile_pool(name="psumSG", bufs=1, space="PSUM"))

        # ---- input DMAs in arrival order (SP queue) ----
        id16_sb = pc.tile([P, P], b16, tag="id16")
        nc.sync.dma_start(id16_sb[:], id16_d.ap())
        x_bf = pc.tile([P, TT, H], b16, tag="xb")
        nc.sync.dma_start(x_bf[:], xr)
        d_sb = pc.tile([P, TT, DCOLS], b16, tag="D")
        nc.sync.dma_start(d_sb[:], dr)
        wgu_sb = pc.tile([P, HT, 704], b16, tag="wgu")
        nc.sync.dma_start(wgu_sb[:], wgu_d.ap())
        wdn_sb = pc.tile([P, 3, H], b16, tag="wdn")
        nc.sync.dma_start(wdn_sb[:], wdn_d.ap())
        wct_sb = pc.tile([P, NCB, T], b16, tag="wct")
        nc.sync.dma_start(wct_sb[:], wct_d.ap())

        # ---- weight stream: consumption-order chunk list with lookahead ----
        stream = []
        for j in range(4):
            stream += [("w13", j, i) for i in range(ITL)]
            stream += [("w2", j, hc) for hc in range(8)]
        BUFS = {"w13": 5, "w2": 3}
        tiles = {}
        outst = {"w13": 0, "w2": 0}
        nxt = [0]

        def issue_avail():
            # issue in stream order while pool capacity is free, so a full
            # pool never parks a blocking DMA at the queue head
            while nxt[0] < len(stream):
                kind, j, i = stream[nxt[0]]
                if outst[kind] >= BUFS[kind]:
                    return
                if kind == "w13":
                    tl = w13sp.tile([P, HT * 2 * P], b16, tag="w13c",
                                    name="w13c")
                    nc.sync.dma_start(
                        tl[:],
                        w13_d.ap()[j][:, i * HT * 2 * P:(i + 1) * HT * 2 * P])
                else:
                    tl = w2sp.tile([P, ITL * 2 * P], b16, tag="w2c",
                                   name="w2c")
                    nc.sync.dma_start(
                        tl[:],
                        w2_d.ap()[j][:, i * ITL * 2 * P:(i + 1) * ITL * 2 * P])
                tiles[nxt[0]] = tl
                outst[kind] += 1
                nxt[0] += 1

        issue_avail()
        consumed = [0]

        def take():
            n = consumed[0]
            tl = tiles.pop(n)
            consumed[0] = n + 1
            outst[stream[n][0]] -= 1
            issue_avail()
            return tl

        # ---- persistent activations ----
        xT = pc.tile([P, HT, T], b16, tag="xT")
        xeT = pc.tile([P, HT, DCOLS], b16, tag="xeT")
        actShT = pc.tile([P, 3, T], b16, tag="actShT")
        peS = pc.tile([P, TT, H], b16, tag="peS")
        ye = pc.tile([P, YRING, H], b16, tag="ye")
        yeT = pc.tile([P, HT, NCB * P], b16, tag="yeT")
        if NCB * P > DCOLS:
            nc.gpsimd.memset(yeT[:, :, DCOLS:], 0.0)

        cpi = [0]

        def cp(out, in_):
            if cpi[0] % 2 == 0:
                nc.vector.tensor_copy(out=out, in_=in_)
            else:
                nc.scalar.copy(out, in_)
            cpi[0] += 1

        # ---- head: x transposes -> xT ----
        for s in range(HT // 2):
            for tt in range(TT):
                pt = psS.tile([P, 2 * P], b16, tag="sm", name="pt_x")
                for hl in range(2):
                    nc.tensor.transpose(
                        pt[:, hl * P:(hl + 1) * P],
                        x_bf[:, tt, (2 * s + hl) * P:(2 * s + hl + 1) * P],
                        id16_sb[:])
                cp(xT[:, 2 * s:2 * s + 2, tt * P:(tt + 1) * P],
                   pt.rearrange("p (k c) -> p k c", k=2))

        # ---- head: dispatch -> xeT ----
        for ko in range(HT):
            pg = psG.tile([P, 512], f32, tag="g", name="pg_d")
            pu = psU.tile([P, 512], f32, tag="u", name="pu_d")
            for tt in range(TT):
                nc.tensor.matmul(pg[:, :cw0], x_bf[:, tt, ko * P:(ko + 1) * P],
                                 d_sb[:, tt, :cw0],
                                 start=(tt == 0), stop=(tt == TT - 1))
                nc.tensor.matmul(pu[:, :cw1], x_bf[:, tt, ko * P:(ko + 1) * P],
                                 d_sb[:, tt, cw0:DCOLS],
                                 start=(tt == 0), stop=(tt == TT - 1))
            cp(xeT[:, ko, :cw0], pg[:, :cw0])
            cp(xeT[:, ko, cw0:DCOLS], pu[:, :cw1])

        # ---- woven PE work: shared-expert chains + combine passes ----
        pending = deque()

        def weave(k=1):
            for _ in range(k):
                if pending:
                    pending.popleft()()

        def shared_pair(mb):
            # 4 pieces of ~1.7us: gate/up chains each split into column halves
            gcol, w = SSEG[mb]
            hold = {}

            def chain(ps, c0, c1, wofs):
                for k in range(HT):
                    nc.tensor.matmul(ps[:w, c0:c1],
                                     wgu_sb[:, k, gcol + wofs:gcol + wofs + w],
                                     xT[:, k, c0:c1],
                                     start=(k == 0), stop=(k == HT - 1))

            def g_half(half):
                if half == 0:
                    hold["sg"] = psSG.tile([P, 512], f32, tag="sg", name="ps_g")
                chain(hold["sg"], half * 256, half * 256 + 256, 0)
                if half == 1:
                    tmp = tp_.tile([P, 512], b16, tag="sgtmp", name="sgtmp",
                                   bufs=1)
                    nc.scalar.activation(tmp[:w, :], hold["sg"][:w, :], Act.Silu)
                    hold["tmp"] = tmp

            def u_half(half):
                if half == 0:
                    hold["su"] = psSG.tile([P, 512], f32, tag="su", name="ps_u")
                chain(hold["su"], half * 256, half * 256 + 256, w)
                if half == 1:
                    nc.vector.tensor_tensor(actShT[:w, mb, :],
                                            hold["tmp"][:w, :],
                                            hold["su"][:w, :], Alu.mult)

            pending.append(lambda: g_half(0))
            pending.append(lambda: g_half(1))
            pending.append(lambda: u_half(0))
            pending.append(lambda: u_half(1))

        pass_first = [True]
        cpool = [0]

        def cpiece(tt, hq, cbs, with_shared, first, last):
            # final pieces rotate through 4 psum tags (sg/su are idle by then)
            rot = ((psG, "g"), (psU, "u"), (psSG, "sg"), (psSG, "su")) \
                if last else ((psG, "g"), (psU, "u"))
            pool, tag = rot[cpool[0] % len(rot)]
            cpool[0] += 1
            po = pool.tile([P, 512], f32, tag=tag, name="po")
            nmm = (3 if with_shared else 0) + len(cbs)
            q = 0
            if with_shared:
                for s in range(3):
                    r = min(P, ISH - s * P)
                    nc.tensor.matmul(
                        po[:], actShT[:r, s, tt * P:(tt + 1) * P],
                        wdn_sb[:r, s, hq * 512:(hq + 1) * 512],
                        start=(q == 0), stop=(q == nmm - 1))
                    q += 1
            for cb in cbs:
                nc.tensor.matmul(
                    po[:], wct_sb[:, cb, tt * P:(tt + 1) * P],
                    ye[:, cb % YRING, hq * 512:(hq + 1) * 512],
                    start=(q == 0), stop=(q == nmm - 1))
                q += 1
            dst = peS[:, tt, hq * 512:(hq + 1) * 512]
            if first:
                nc.vector.tensor_copy(out=dst, in_=po[:])
            elif not last:
                nc.vector.tensor_tensor(dst, dst, po[:], Alu.add)
            else:
                stg = tp_.tile([P, 512], b16, tag="stg", name="stg", bufs=4)
                nc.vector.tensor_tensor(stg[:], dst, po[:], Alu.add)
                if single_core:
                    dd = out_d.ap()[tt * P:(tt + 1) * P,
                                    hq * 512:(hq + 1) * 512]
                else:
                    dd = partial_d.ap()[tt * P:(tt + 1) * P,
                                        hq * 512:(hq + 1) * 512]
                nc.gpsimd.dma_start(dd, stg[:])

        def combine_pass(cbs, with_shared):
            if not cbs and not with_shared:
                return
            first = pass_first[0]
            for tt in range(TT):
                for hq in range(4):
                    pending.append(
                        lambda tt=tt, hq=hq, cbs=tuple(cbs), ws=with_shared,
                        first=first: cpiece(tt, hq, cbs, ws, first, False))
            pass_first[0] = False

        # ---- expert pipeline ----
        cb_done = [0]

        def ye_transpose(cb, s):
            pt = psS.tile([P, 2 * P], b16, tag="sm", name="pt_ye")
            for hl in range(2):
                nc.tensor.transpose(
                    pt[:, hl * P:(hl + 1) * P],
                    yeT[:, 2 * s + hl, cb * P:(cb + 1) * P], id16_sb[:])
            cp(ye[:, cb % YRING, 2 * s * P:(2 * s + 2) * P],
               pt.rearrange("p (k c) -> p k c", k=2))

        def expert(j):
            cap = slot_caps[j]
            off = offs[j]
            last = (j == 3)
            if j < 3:
                shared_pair(j)
            act = ap_.tile([P, ITL, CAPMAX], b16, tag="act", name="act")
            for i in range(ITL):
                wg = take().rearrange("p (k f) -> p k f", f=2 * P)
                pg = psG.tile([P, 512], f32, tag="g", name="pg_w13")
                pu = psU.tile([P, 512], f32, tag="u", name="pu_w13")
                for k in range(HT):
                    nc.tensor.matmul(pg[:, :cap], wg[:, k, :P],
                                     xeT[:, k, off:off + cap],
                                     start=(k == 0), stop=(k == HT - 1))
                    nc.tensor.matmul(pu[:, :cap], wg[:, k, P:2 * P],
                                     xeT[:, k, off:off + cap],
                                     start=(k == 0), stop=(k == HT - 1))
                tmp = tp_.tile([P, 512], b16, tag="silu", name="silu")
                nc.scalar.activation(tmp[:, :cap], pg[:, :cap], Act.Silu)
                nc.vector.tensor_tensor(act[:, i, :cap], tmp[:, :cap],
                                        pu[:, :cap], Alu.mult)
                weave(2 if last else 1)
            hi = (offs[j + 1] // P) if j < 3 else NCB
            cbs = list(range(cb_done[0], hi))
            cb_done[0] = hi
            if last:
                while pending:       # all earlier peS adds must precede stg
                    pending.popleft()()
            for hc in range(8):
                w2c = take().rearrange("p (k f) -> p k f", f=2 * P)
                for m in range(2):
                    pool = psG if m == 0 else psU
                    py = pool.tile([P, 512], f32, tag="g" if m == 0 else "u",
                                   name="py")
                    for ki in range(ITL):
                        nc.tensor.matmul(py[:, :cap], w2c[:, ki, m * P:(m + 1) * P],
                                         act[:, ki, :cap],
                                         start=(ki == 0), stop=(ki == ITL - 1))
                    cp(yeT[:, 2 * hc + m, off:off + cap], py[:, :cap])
                if not last:
                    weave(1)
                else:
                    # h-interleaved finish: transpose this h-pair for the
                    # final c-blocks, then emit the output pieces for the
                    # completed h-quarter (overlaps the DMA stream tail)
                    for cb in cbs:
                        ye_transpose(cb, hc)
                    if hc % 2 == 1:
                        hq = hc // 2
                        for tt in range(TT):
                            cpiece(tt, hq, cbs, False, False, True)
            if not last:
                for cb in cbs:
                    for s in range(HT // 2):
                        ye_transpose(cb, s)
                        if s % 4 == 3:
                            weave(1)
                combine_pass(cbs, with_shared=(j == 2))

        for j in range(4):
            expert(j)
        while pending:
            pending.popleft()()

        if not single_core:
            nc.gpsimd.collective_compute(
                "ReduceScatter", Alu.add,
                replica_groups=[list(range(NCORES))],
                ins=[partial_d.ap().opt()],
                outs=[rs_d.ap().opt()],
            )
            nc.sync.dma_start(out_d.ap(), rs_d.ap())

    nc.compile()
    return nc


_NC_CACHE = {}


def _prepare(hidden_states, gate_w, bias):
    x = np.ascontiguousarray(np.asarray(hidden_states, np.float32))
    weights, topk = _host_route(x, np.ascontiguousarray(np.asarray(gate_w, np.float32)),
                                np.asarray(bias, np.float32))
    groups, slot_caps = _plan(topk)
    return x, weights, topk, groups, slot_caps


def kernel(hidden_states, residual, gate_w, bias, w13, w2, shared_gate_up,
           shared_down):
    from concourse.bass_utils import run_bass_kernel_spmd

    x, weights, topk, groups, slot_caps = _prepare(hidden_states, gate_w, bias)
    w13 = np.asarray(w13, np.float32)
    w2 = np.asarray(w2, np.float32)
    sgu = np.asarray(shared_gate_up, np.float32)
    sdn = np.asarray(shared_down, np.float32)

    Ds, WcTs, offs, DCOLS, NCB = _build_dispatch(weights, topk, groups,
                                                 slot_caps)

    key = tuple(slot_caps)
    if key not in _NC_CACHE:
        _NC_CACHE[key] = _build_nc(slot_caps)
    nc = _NC_CACHE[key]

    id16 = np.eye(P, dtype=np.float32).astype(bf16)
    x_bf = x.astype(bf16)
    in_maps = []
    for core in range(NCORES):
        in_maps.append({
            "x_bf": x_bf,
            "disp": Ds[core].astype(bf16),
            "wct": WcTs[core].astype(bf16),
            "w13p": np.stack([_pack_w13(w13[e]).reshape(P, -1)
                              for e in groups[core]]),
            "w2p": np.stack([_pack_w2(w2[e]).reshape(P, -1)
                             for e in groups[core]]),
            "wgup": _pack_wgu(sgu, core),
            "wdnp": _pack_wdn(sdn, core),
            "id_b16": id16,
        })
    res = run_bass_kernel_spmd(nc, in_maps, core_ids=list(range(NCORES)))
    out = np.concatenate([np.asarray(res.results[c]["out_slice"])
                          for c in range(NCORES)], axis=0)
    return out.astype(np.float32)
